# revision 12
# baseline (speedup 1.0000x reference)
"""MoE-GRN kernel for Trainium2, 8 NeuronCores, data-parallel over batch.

Reference computation (B=4096, IN=1024, J=HID*E=16384, Dtot=OUT*E=8192, E=8,
C=1000, TOPK=2):
    gate_logits = x @ Wg.T + bg                     [B, E]
    Gx = ||gate_logits||_2 per row; Nx = Gx / (mean_B(Gx) + 1e-6)
    gate_probs = softmax(gamma * (gate_logits * Nx) + beta)
    topk over E=8 (k=2)
    h  = relu(x @ W1.T + b1)                        [B, J]
    eo = (h @ W2.T + b2).reshape(B, E, OUT)
    out = sum_k topk_probs * eo[topk_idx]           [B, OUT]
    y  = out @ Wc.T + bc                            [B, C]

Sharding: batch split 8 ways (512 tokens/core), weights replicated.  The GRN
batch-mean couples all tokens, so every core recomputes the (tiny) full-batch
gate logits in fp32 to derive mean(Gx) locally — no collectives.

Per core the MLP runs transposed (hT layout) so no on-device transposes are
needed anywhere: host pre-transposes/pre-tiles x and the weights.  fc1/fc2 run
in float32r (full PE rate, ~1e-3 rel err); gating + classifier in fp32.

Program order is tuned so the PE never starves: fc1 of split 0 is emitted
before gating (its DMAs land first), the batch-mean reduction/broadcast runs
on the PE (ones-matmuls) instead of GpSimd, and fc1/fc2 share one 8-buffer
PSUM pool so split boundaries hand banks over smoothly.
"""

import numpy as np

import concourse.bass as bass
import concourse.mybir as mybir
import concourse.tile as tile
from concourse import bacc
from concourse.bass_utils import run_bass_kernel_spmd

F32 = mybir.dt.float32
F32R = mybir.dt.float32r
AF = mybir.ActivationFunctionType
ALU = mybir.AluOpType

B, IN, J, DTOT, E, C = 4096, 1024, 16384, 8192, 8, 1000
NCORES = 8
T = B // NCORES            # 512 tokens per core
TS = T // 128              # 4 token subtiles
ITS = IN // 128            # 8 k-subtiles over IN
NSPLIT = 8                 # J split into 8 chunks of 2048
JT_PER_S = J // NSPLIT // 128   # 16 j-tiles per split
NDT = DTOT // 128          # 64 d-tiles
DDT = 1024 // 128          # 8 d-subtiles per expert block
NTT = B // 128             # 32 full-batch token tiles
EPS = 1e-6


def _build(flags):
    has_bg, has_gb, has_b2, has_bc = (
        flags["bg"], flags["gb"], flags["b2"], flags["bc"])
    nc = bacc.Bacc("TRN2", target_bir_lowering=False)

    # ---- DRAM I/O ----
    xf_d = nc.dram_tensor("xf", [NTT, 128, ITS, 128], F32, kind="ExternalInput")
    xg_d = nc.dram_tensor("xg", [128, ITS, T], F32, kind="ExternalInput")
    xs_d = nc.dram_tensor("xs", [128, ITS, T], F32R, kind="ExternalInput")
    wg_d = nc.dram_tensor("wg", [128, ITS, E], F32, kind="ExternalInput")
    w1_d = nc.dram_tensor("w1", [J // 128, 128, ITS, 128], F32R, kind="ExternalInput")
    w2_d = nc.dram_tensor("w2", [NSPLIT, NDT, 128, JT_PER_S, 128], F32R,
                          kind="ExternalInput")
    wc_d = nc.dram_tensor("wc", [128, DDT, C], F32, kind="ExternalInput")
    b1_d = nc.dram_tensor("b1s", [128, J // 128], F32, kind="ExternalInput")
    if has_bg:
        bg_d = nc.dram_tensor("bgb", [128, E], F32, kind="ExternalInput")
    if has_gb:
        ga_d = nc.dram_tensor("gammab", [128, E], F32, kind="ExternalInput")
        be_d = nc.dram_tensor("betab", [128, E], F32, kind="ExternalInput")
    if has_b2:
        b2_d = nc.dram_tensor("b2s", [128, NDT], F32, kind="ExternalInput")
    if has_bc:
        bc_d = nc.dram_tensor("bcr", [1, C], F32, kind="ExternalInput")
    out_d = nc.dram_tensor("out", [TS, 128, C], F32, kind="ExternalOutput")

    with tile.TileContext(nc) as tc:
        with tc.tile_pool(name="const", bufs=1) as cp, \
             tc.tile_pool(name="dram", bufs=1, space="DRAM") as dp, \
             tc.tile_pool(name="ps", bufs=6, space="PSUM") as psp:
            # resident tiles needed by fc1 first (their DMAs go first)
            xs = cp.tile([128, ITS, T], F32R, tag="xs")
            nc.sync.dma_start(xs[:], xs_d[:])
            b1s = cp.tile([128, J // 128], F32, tag="b1s")
            nc.sync.dma_start(b1s[:], b1_d[:])
            split_pools = tc.tile_pool(name="hqp", bufs=1)
            hqp = split_pools.__enter__()
            w1p_cm = tc.tile_pool(name="w1p", bufs=3)
            w1p = w1p_cm.__enter__()
            w2p_cm = tc.tile_pool(name="w2p", bufs=6)
            w2p = w2p_cm.__enter__()

            def fc1_split(s, hq):
                for jtl in range(JT_PER_S):
                    jt = s * JT_PER_S + jtl
                    w1t = w1p.tile([128, ITS, 128], F32R, tag="w1t")
                    nc.sync.dma_start(w1t[:], w1_d[jt])
                    ph = psp.tile([128, T], F32, tag="ps_shared")
                    for it in range(ITS):
                        nc.tensor.matmul(ph[:], w1t[:, it, :], xs[:, it, :],
                                         start=(it == 0), stop=(it == ITS - 1))
                    nc.scalar.activation(hq[:, jtl, :], ph[:], AF.Relu,
                                         bias=b1s[:, jt:jt + 1])

            # ---- split 0 fc1 first: PE has dense work from the start ----
            hq0 = hqp.tile([128, JT_PER_S, T], F32R, tag="hq")
            fc1_split(0, hq0)

            # ---- gating (fp32, exact) ----
            xg = cp.tile([128, ITS, T], F32, tag="xg")
            nc.sync.dma_start(xg[:], xg_d[:])
            wg = cp.tile([128, ITS, E], F32, tag="wg")
            nc.sync.dma_start(wg[:], wg_d[:])
            if has_bg:
                bgb = cp.tile([128, E], F32, tag="bgb")
                nc.sync.dma_start(bgb[:], bg_d[:])
            if has_gb:
                gab = cp.tile([128, E], F32, tag="gammab")
                nc.sync.dma_start(gab[:], ga_d[:])
                beb = cp.tile([128, E], F32, tag="betab")
                nc.sync.dma_start(beb[:], be_d[:])
            ones1 = cp.tile([1, 128], F32, tag="ones1")
            nc.any.memset(ones1[:], 1.0)
            ones_c = cp.tile([128, 1], F32, tag="ones_c")
            nc.any.memset(ones_c[:], 1.0)

            moe = cp.tile([128, DDT, T], F32, tag="moe")     # combined eoT
            nc.any.memset(moe[:], 0.0)
            wb = cp.tile([128, E, T], F32, tag="wb")         # bcast top2 weights
            w_all = cp.tile([128, TS, E], F32, tag="w_all")  # per-token weights

            with tc.tile_pool(name="gin", bufs=3) as gin, \
                 tc.tile_pool(name="gtmp", bufs=4) as gt, \
                 tc.tile_pool(name="gps", bufs=2, space="PSUM") as gps:
                ss_all = cp.tile([128, NTT], F32, tag="ss_all")
                # full-batch squared row norms of gate logits
                for tt in range(NTT):
                    xt = gin.tile([128, ITS, 128], F32, tag="xf_t")
                    nc.sync.dma_start(xt[:], xf_d[tt])
                    pg = gps.tile([128, E], F32, tag="pg")
                    for it in range(ITS):
                        nc.tensor.matmul(pg[:], xt[:, it, :], wg[:, it, :],
                                         start=(it == 0), stop=(it == ITS - 1))
                    if has_bg:
                        lg = gt.tile([128, E], F32, tag="lg")
                        nc.vector.tensor_add(lg[:], pg[:], bgb[:])
                        src = lg
                    else:
                        src = pg
                    sq = gt.tile([128, E], F32, tag="sq")
                    nc.scalar.square(sq[:], src[:])
                    nc.vector.reduce_sum(ss_all[:, tt:tt + 1], sq[:],
                                         axis=mybir.AxisListType.X)
                gx_all = gt.tile([128, NTT], F32, tag="gx_all")
                nc.scalar.activation(gx_all[:], ss_all[:], AF.Sqrt)
                gsum = gt.tile([128, 1], F32, tag="gsum")
                nc.vector.reduce_sum(gsum[:], gx_all[:], axis=mybir.AxisListType.X)
                # partition-sum + mean + reciprocal + partition-broadcast, all
                # via tiny PE matmuls (keeps GpSimd off the critical path)
                ptot = gps.tile([128, E], F32, tag="pg")
                nc.tensor.matmul(ptot[:1, :1], ones_c[:], gsum[:],
                                 start=True, stop=True)
                t1 = gt.tile([1, 1], F32, tag="t1")
                nc.vector.tensor_scalar(t1[:], ptot[:1, :1], 1.0 / B, EPS,
                                        op0=ALU.mult, op1=ALU.add)
                rec1 = gt.tile([1, 1], F32, tag="rec1")
                nc.vector.reciprocal(rec1[:], t1[:])
                pbc = gps.tile([128, E], F32, tag="pg")
                nc.tensor.matmul(pbc[:, :1], ones1[:], rec1[:],
                                 start=True, stop=True)
                nxs = gt.tile([128, 1], F32, tag="nxs")
                nc.scalar.copy(nxs[:], pbc[:, :1])

                # shard gating -> top2-masked prob weights w_all; each st's
                # weights bounce through DRAM immediately (partition->free
                # transpose) so wb is ready soon after the last st.
                wdr = dp.tile([E, TS, 128], F32, tag="wdr")
                for st in range(TS):
                    pgs = gps.tile([128, E], F32, tag="pg")
                    for it in range(ITS):
                        nc.tensor.matmul(pgs[:],
                                         xg[:, it, st * 128:(st + 1) * 128],
                                         wg[:, it, :],
                                         start=(it == 0), stop=(it == ITS - 1))
                    lgs = gt.tile([128, E], F32, tag="lgs")
                    if has_bg:
                        nc.vector.tensor_add(lgs[:], pgs[:], bgb[:])
                    else:
                        nc.scalar.copy(lgs[:], pgs[:])
                    sq = gt.tile([128, E], F32, tag="sq")
                    nc.scalar.square(sq[:], lgs[:])
                    ss1 = gt.tile([128, 1], F32, tag="ss1")
                    nc.vector.reduce_sum(ss1[:], sq[:], axis=mybir.AxisListType.X)
                    gx1 = gt.tile([128, 1], F32, tag="gx1")
                    nc.scalar.activation(gx1[:], ss1[:], AF.Sqrt)
                    nx = gt.tile([128, 1], F32, tag="nx")
                    nc.vector.tensor_mul(nx[:], gx1[:], nxs[:])
                    mod = gt.tile([128, E], F32, tag="mod")
                    nc.vector.tensor_scalar_mul(mod[:], lgs[:], nx[:])
                    if has_gb:
                        nc.vector.tensor_mul(mod[:], mod[:], gab[:])
                        nc.vector.tensor_add(mod[:], mod[:], beb[:])
                    rmax = gt.tile([128, 1], F32, tag="rmax")
                    nc.vector.reduce_max(rmax[:], mod[:], axis=mybir.AxisListType.X)
                    nrm = gt.tile([128, 1], F32, tag="nrm")
                    nc.vector.tensor_scalar_mul(nrm[:], rmax[:], -1.0)
                    ex = gt.tile([128, E], F32, tag="ex")
                    nc.scalar.activation(ex[:], mod[:], AF.Exp, bias=nrm[:])
                    sm = gt.tile([128, 1], F32, tag="sm")
                    nc.vector.reduce_sum(sm[:], ex[:], axis=mybir.AxisListType.X)
                    rs = gt.tile([128, 1], F32, tag="rs")
                    nc.vector.reciprocal(rs[:], sm[:])
                    probs = gt.tile([128, E], F32, tag="probs")
                    nc.vector.tensor_scalar_mul(probs[:], ex[:], rs[:])
                    mx8 = gt.tile([128, 8], F32, tag="mx8")
                    nc.vector.max(mx8[:], probs[:])
                    msk = gt.tile([128, E], F32, tag="msk")
                    nc.vector.tensor_scalar(msk[:], probs[:], mx8[:, 1:2], None,
                                            op0=ALU.is_ge)
                    nc.vector.tensor_mul(w_all[:, st, :], msk[:], probs[:])
                    nc.sync.dma_start(wdr[:, st, :].rearrange("e p -> p e"),
                                      w_all[:, st, :])

                # read back transposed + partition-broadcast via ones-matmul
                wrows = cp.tile([1, E, T], F32, tag="wrows")
                nc.sync.dma_start(wrows[:],
                                  wdr[:].rearrange("e s p -> e (s p)")[None])
                for e in range(E):
                    pwb = psp.tile([128, T], F32, tag="ps_shared")
                    nc.tensor.matmul(pwb[:], ones1[:], wrows[:, e, :],
                                     start=True, stop=True)
                    nc.scalar.copy(wb[:, e, :], pwb[:])

            # ---- fc2 split 0, then fc1+fc2 for splits 1..7 ----
            def fc2_split(s, hq):
                for dt_ in range(NDT):
                    w2t = w2p.tile([128, JT_PER_S, 128], F32R, tag="w2t")
                    nc.sync.dma_start(w2t[:], w2_d[s, dt_])
                    pe_ = psp.tile([128, T], F32, tag="ps_shared")
                    for ktl in range(JT_PER_S):
                        nc.tensor.matmul(pe_[:], w2t[:, ktl, :], hq[:, ktl, :],
                                         start=(ktl == 0),
                                         stop=(ktl == JT_PER_S - 1))
                    if has_b2 and s == 0:
                        nc.scalar.activation(pe_[:], pe_[:], AF.Identity,
                                             bias=b2s[:, dt_:dt_ + 1])
                    e = dt_ // DDT
                    ddt = dt_ % DDT
                    nc.vector.tensor_mul(pe_[:], pe_[:], wb[:, e, :])
                    nc.vector.tensor_add(moe[:, ddt, :], moe[:, ddt, :], pe_[:])

            if has_b2:
                b2s = cp.tile([128, NDT], F32, tag="b2s")
                nc.sync.dma_start(b2s[:], b2_d[:])
            fc2_split(0, hq0)
            for s in range(1, NSPLIT):
                hq = hqp.tile([128, JT_PER_S, T], F32R, tag="hq")
                fc1_split(s, hq)
                fc2_split(s, hq)

            w2p_cm.__exit__(None, None, None)
            w1p_cm.__exit__(None, None, None)
            split_pools.__exit__(None, None, None)

            # ---- classifier (fp32) ----
            with tc.tile_pool(name="clsp", bufs=1) as clp, \
                 tc.tile_pool(name="outp", bufs=2) as outp:
                wc = clp.tile([128, DDT, C], F32, tag="wc")
                nc.sync.dma_start(wc[:], wc_d[:])
                if has_bc:
                    bct = clp.tile([1, C], F32, tag="bcr")
                    nc.sync.dma_start(bct[:], bc_d[:])
                for st in range(TS):
                    ot = outp.tile([128, C], F32, tag="ot")
                    for c0, cw in ((0, 512), (512, C - 512)):
                        pc = psp.tile([128, T], F32, tag="ps_shared")
                        for kt in range(DDT):
                            nc.tensor.matmul(
                                pc[:, :cw],
                                moe[:, kt, st * 128:(st + 1) * 128],
                                wc[:, kt, c0:c0 + cw],
                                start=(kt == 0),
                                stop=(kt == DDT - 1 and not has_bc))
                        if has_bc:
                            nc.tensor.matmul(pc[:, :cw], ones1[:],
                                             bct[:, c0:c0 + cw],
                                             start=False, stop=True)
                        nc.scalar.copy(ot[:, c0:c0 + cw], pc[:, :cw])
                    nc.sync.dma_start(out_d[st], ot[:])

    nc.compile()
    return nc


_CACHE = {}


def _get_program(flags):
    key = tuple(sorted(flags.items()))
    if key not in _CACHE:
        _CACHE[key] = _build(flags)
    return _CACHE[key]


def _prep_inputs(x, Wg, bg, gamma, beta, W1, b1, W2, b2, Wc, bc):
    f = np.float32
    a = np.ascontiguousarray
    x = np.asarray(x, f)
    flags = {
        "bg": bool(np.any(np.asarray(bg))),
        "gb": bool(np.any(np.asarray(gamma) != 1.0) or np.any(np.asarray(beta))),
        "b2": bool(np.any(np.asarray(b2))),
        "bc": bool(np.any(np.asarray(bc))),
    }
    shared = {
        "xf": a(x.reshape(NTT, 128, ITS, 128).transpose(0, 3, 2, 1)),
        "wg": a(np.asarray(Wg, f).reshape(E, ITS, 128).transpose(2, 1, 0)),
        "w1": a(np.asarray(W1, f).reshape(J // 128, 128, ITS, 128)
                .transpose(0, 3, 2, 1)),
        "w2": a(np.asarray(W2, f).reshape(NDT, 128, NSPLIT, JT_PER_S, 128)
                .transpose(2, 0, 4, 3, 1)),
        "wc": a(np.asarray(Wc, f).reshape(C, DDT, 128).transpose(2, 1, 0)),
        "b1s": a(np.asarray(b1, f).reshape(J // 128, 128).T),
    }
    if flags["bg"]:
        shared["bgb"] = a(np.broadcast_to(np.asarray(bg, f).reshape(1, E),
                                          (128, E)))
    if flags["gb"]:
        shared["gammab"] = a(np.broadcast_to(np.asarray(gamma, f).reshape(1, E),
                                             (128, E)))
        shared["betab"] = a(np.broadcast_to(np.asarray(beta, f).reshape(1, E),
                                            (128, E)))
    if flags["b2"]:
        shared["b2s"] = a(np.asarray(b2, f).reshape(NDT, 128).T)
    if flags["bc"]:
        shared["bcr"] = a(np.asarray(bc, f).reshape(1, C))
    in_maps = []
    for c in range(NCORES):
        xsh = a(x[c * T:(c + 1) * T].reshape(T, ITS, 128).transpose(2, 1, 0))
        m = dict(shared)
        m["xg"] = xsh
        m["xs"] = xsh
        in_maps.append(m)
    return flags, in_maps


def _run(inputs, trace=False):
    flags, in_maps = _prep_inputs(**inputs)
    nc = _get_program(flags)
    res = run_bass_kernel_spmd(nc, in_maps, core_ids=list(range(NCORES)),
                               trace=trace)
    out = np.concatenate(
        [res.results[c]["out"].reshape(T, C) for c in range(NCORES)], axis=0)
    return out, res


def kernel(**inputs) -> np.ndarray:
    out, _ = _run(inputs, trace=False)
    return out
